# revision 18
# baseline (speedup 1.0000x reference)
"""AIMv2 attention (B=4, S=2048, D=1024, H=16, d=64) on 8 TRN2 NeuronCores.

Sharding: core c = (batch b = c//2, head-group g = c%2 of 8 heads).
Each core computes its batch's attention for its 8 heads plus the
out-projection partial sum over its heads' rows of w_out; the host adds
the two partials per batch (no on-device collectives needed).

Per-core kernel (all matmuls in bf16, fp32 accumulation):
  X^T via DVE cast + batched XBAR DMA transposes; Q^T,K^T = Wq/k^T @ X^T
  so the score matmuls produce s_T[k, q] directly with head pairs in
  row-groups (even head partitions 0-63, odd 64-127); softmax without
  max-subtraction (scores ~ N(0,1), exp never overflows); V carries a
  ones column so ctx' = [V|1]^T @ P^T yields both ctx^T and the softmax
  denominators in one accumulation.

  The attention inner loop is a lag-3 software pipeline: ctx matmuls for
  k-tile kt are emitted alongside scores for kt+3, so TensorE never
  waits on the ScalarE exp stream (the kernel is exp-throughput bound).
  Cross-pair cleanup (last ctx tiles, PSUM evacuation, normalization)
  drains during the first k-tiles of the next pair.
"""

import numpy as np

import concourse.bass as bass
import concourse.tile as tile
from concourse import bacc, mybir
from concourse.bass_utils import run_bass_kernel_spmd

P = 128
S = 2048          # sequence length
D = 1024          # model dim
DQ = 512          # per-core qkv width (8 heads x 64)
HD = 64           # head dim
NH = 8            # heads per core
NKT = D // P      # 8 contraction tiles over D
NST = S // P      # 16 tiles over S
QC = 1024         # q chunk for attention inner loop
LAG = 4           # ctx matmul lag behind scores/exp in the pipeline
SCALE = 1.0 / 8.0  # 1/sqrt(64)

F32 = mybir.dt.float32
BF16 = mybir.dt.bfloat16


def build_kernel(nc, out_ap, hs_ap, wqkv_ap, wout_ap):
    import contextlib

    ctx = contextlib.ExitStack()
    with tile.TileContext(nc) as tc:
        with ctx:
            _body(ctx, tc, nc, out_ap, hs_ap, wqkv_ap, wout_ap)


def _body(ctx, tc, nc, out_ap, hs_ap, wqkv_ap, wout_ap):
    Exp = mybir.ActivationFunctionType.Exp

    persist = ctx.enter_context(tc.tile_pool(name="persist", bufs=1))
    psum = ctx.enter_context(tc.tile_pool(name="psum", bufs=1, space="PSUM"))

    # all-ones [128, 64] so a ones-row lhsT can be sliced at any base
    # partition (matmul requires lhsT/rhs base partitions to match)
    ones_rows = persist.tile([P, HD], BF16, name="ones_rows")
    nc.vector.memset(ones_rows[:], 1.0)

    wout_bf = []
    vc = [persist.tile([P, NH, HD + 1], BF16, name=f"vc{st}") for st in range(NST)]
    qt = [persist.tile([P, S], BF16, name=f"qt{m}") for m in range(4)]
    kt_sb = [persist.tile([P, S], BF16, name=f"kt{m}") for m in range(4)]
    ctxt = [persist.tile([P, S], BF16, name=f"ctxt{m}") for m in range(4)]

    # ================= head: loads + all projections =================
    # X^T and the qkv weights are only needed for the projections; scope
    # them so their SBUF goes back before the attention pools open.
    with tc.tile_pool(name="proj_scope", bufs=1) as pscope:
        xt3 = pscope.tile([P, NKT, S], BF16, name="xt3")
        wqkv_bf = []
        with tc.tile_pool(name="stage", bufs=3) as stage:
            for kt in range(NKT):
                wf = stage.tile([P, 3 * DQ], F32, tag="wstage", bufs=2)
                nc.gpsimd.dma_start(wf[:], wqkv_ap[kt * P:(kt + 1) * P, :])
                wb = pscope.tile([P, 3 * DQ], BF16, name=f"wqkv_bf{kt}")
                nc.scalar.copy(wb[:], wf[:])
                wqkv_bf.append(wb)

            for i in range(DQ // P):
                wf = stage.tile([P, D], F32, tag="wostage", bufs=2)
                nc.gpsimd.dma_start(wf[:], wout_ap[i * P:(i + 1) * P, :])
                wb = persist.tile([P, D], BF16, name=f"wout_bf{i}")
                nc.scalar.copy(wb[:], wf[:])
                wout_bf.append(wb)

            warm_ps = psum.tile([HD, HD], F32, tag="ctxA", bufs=1, name="warm_ps")
            n_warm = 2 * NST
            wi = 0
            def warm(src_tile):
                nonlocal wi
                nc.tensor.matmul(
                    warm_ps[:], lhsT=src_tile[0:HD, 0:HD],
                    rhs=src_tile[0:HD, 0:HD],
                    start=(wi == 0), stop=(wi == n_warm - 1),
                )
                wi += 1

            # X: load, cast bf16, batched XBAR transpose into [D, S] layout
            for st in range(NST):
                xf = stage.tile([P, D], F32, tag="xstage", bufs=5)
                nc.scalar.dma_start(xf[:], hs_ap[st * P:(st + 1) * P, :])
                xb = stage.tile([P, D], BF16, tag="xbf", bufs=4)
                nc.vector.tensor_copy(xb[:], xf[:])
                nc.sync.dma_start_transpose(
                    xt3[:, :, st * P:(st + 1) * P], xb[:]
                )
                warm(xf)
                warm(xb)

        warmsb = pscope.tile([HD, HD], F32, name="warmsb")
        nc.vector.tensor_scalar_mul(warmsb[:], warm_ps[:], 0.0)
        nc.vector.tensor_add(ones_rows[0:HD, :], ones_rows[0:HD, :], warmsb[:])

        def xt(kt):
            return xt3[:, kt, :]

        # V projection with ones column: vc[st][:, h, 0:64]=V_h, [...,64]=1
        for st in range(NST):
            nc.vector.memset(vc[st][:, :, HD:HD + 1], 1.0)
        for stq in range(NST // 2):
            ps = psum.tile([P, 2 * DQ], F32, tag="sc", bufs=2)
            for half in range(2):
                st = 2 * stq + half
                sl = slice(half * DQ, (half + 1) * DQ)
                for kt in range(NKT):
                    nc.tensor.matmul(
                        ps[:, sl],
                        lhsT=xt(kt)[:, st * P:(st + 1) * P],
                        rhs=wqkv_bf[kt][:, 2 * DQ:3 * DQ],
                        start=(kt == 0),
                        stop=(kt == NKT - 1),
                    )
            for half in range(2):
                st = 2 * stq + half
                src = ps[:, half * DQ:(half + 1) * DQ].rearrange(
                    "p (h e) -> p h e", h=NH
                )
                nc.vector.tensor_copy(vc[st][:, :, 0:HD], src)

        # Q^T / K^T projections for all head pairs
        for m in range(4):
            for which, dst in ((0, qt[m]), (DQ, kt_sb[m])):
                for nqq in range(2):
                    ps = psum.tile([P, 2 * 512], F32, tag="sc", bufs=2)
                    for half in range(2):
                        nq = 2 * nqq + half
                        sl = slice(half * 512, (half + 1) * 512)
                        for kt in range(NKT):
                            nc.tensor.matmul(
                                ps[:, sl],
                                lhsT=wqkv_bf[kt][
                                    :, which + m * P: which + (m + 1) * P
                                ],
                                rhs=xt(kt)[:, nq * 512:(nq + 1) * 512],
                                start=(kt == 0),
                                stop=(kt == NKT - 1),
                            )
                    nc.vector.tensor_copy(
                        dst[:, nqq * 1024:(nqq + 1) * 1024], ps[:]
                    )

    # ================= attention =================
    pt_pool = ctx.enter_context(tc.tile_pool(name="pt", bufs=18))
    small = ctx.enter_context(tc.tile_pool(name="small", bufs=4))
    outsb_pool = ctx.enter_context(tc.tile_pool(name="outsb", bufs=3))

    # deferred cross-pair work: closures drained 2-per-k-tile during the
    # first LAG k-tiles of the following pair (while it has no ctx work)
    pending = []

    def drain(n):
        for _ in range(min(n, len(pending))):
            pending.pop(0)()

    def normalize(csb, hp, qc, rows):
        """ctx^T[d,q] /= sum[q] (sums in row 64 of csb)."""
        q0 = qc * QC
        bc = psum.tile([HD, QC], F32, tag="sc", bufs=2)
        for half in range(2):
            sl = slice(half * 512, (half + 1) * 512)
            nc.tensor.matmul(
                bc[:, sl], lhsT=ones_rows[HD:HD + 1, :],
                rhs=csb[HD:HD + 1, sl],
                start=True, stop=True,
            )
        rec = small.tile([HD, QC], F32, tag="rec", bufs=2)
        nc.vector.reciprocal_approx_fast(rec[:], bc[:])
        nc.vector.tensor_mul(
            ctxt[hp][rows, q0:q0 + QC], csb[0:HD, :], rec[:]
        )

    def attend(hp, qc):
        """Heads (2hp, 2hp+1): even head on partitions 0-63, odd on 64-127."""
        q0 = qc * QC
        hA, hB = 2 * hp, 2 * hp + 1
        state = {}

        def emit_scores(kti):
            psA = psum.tile([P, QC], F32, tag="sc", bufs=2)
            psB = psum.tile([P, QC], F32, tag="sc", bufs=2)
            for half in range(2):
                sl = slice(half * 512, (half + 1) * 512)
                qsl = slice(q0 + half * 512, q0 + (half + 1) * 512)
                nc.tensor.matmul(
                    psA[:, sl],
                    lhsT=kt_sb[hp][0:HD, kti * P:(kti + 1) * P],
                    rhs=qt[hp][0:HD, qsl],
                    start=True, stop=True,
                )
                nc.tensor.matmul(
                    psB[:, sl],
                    lhsT=kt_sb[hp][HD:P, kti * P:(kti + 1) * P],
                    rhs=qt[hp][HD:P, qsl],
                    start=True, stop=True,
                )
            return psA, psB

        def emit_exp(psA, psB):
            ptA = pt_pool.tile([P, QC], BF16, tag="pt", bufs=18)
            ptB = pt_pool.tile([P, QC], BF16, tag="pt", bufs=18)
            nc.scalar.activation(ptA[:], psA[:], Exp, scale=SCALE)
            nc.scalar.activation(ptB[:], psB[:], Exp, scale=SCALE)
            return ptA, ptB

        def emit_ctx(kti, ptA, ptB):
            if kti == 0:
                state["ctxA"] = psum.tile([HD + 1, QC], F32, tag="ctxA", bufs=1, name="ctxA")
                state["ctxB"] = psum.tile([HD + 1, QC], F32, tag="ctxB", bufs=1, name="ctxB")
            first = kti == 0
            last = kti == NST - 1
            for half in range(2):
                sl = slice(half * 512, (half + 1) * 512)
                nc.tensor.matmul(
                    state["ctxA"][:, sl], lhsT=vc[kti][:, hA, :],
                    rhs=ptA[:, sl], start=first, stop=last,
                )
                nc.tensor.matmul(
                    state["ctxB"][:, sl], lhsT=vc[kti][:, hB, :],
                    rhs=ptB[:, sl], start=first, stop=last,
                )

        pts = {}
        for kti in range(NST):
            ps = emit_scores(kti)
            if kti < LAG:
                drain(2)           # previous pair's tail work
            else:
                emit_ctx(kti - LAG, *pts.pop(kti - LAG))
            pts[kti] = emit_exp(*ps)

        # tail: last LAG ctx tiles + PSUM evacuation + normalization are
        # deferred into the next pair's first k-tiles
        def tail_ctx(kti):
            def f():
                emit_ctx(kti, *pts.pop(kti))
            return f

        for kti in range(NST - LAG, NST):
            pending.append(tail_ctx(kti))

        def evac():
            csbA = small.tile([HD + 1, QC], BF16, tag="csb", bufs=4)
            nc.vector.tensor_copy(csbA[:], state["ctxA"][:])
            csbB = small.tile([HD + 1, QC], BF16, tag="csb", bufs=4)
            nc.vector.tensor_copy(csbB[:], state["ctxB"][:])
            state["csbA"], state["csbB"] = csbA, csbB

        pending.append(evac)
        pending.append(lambda: normalize(state["csbA"], hp, qc, slice(0, HD)))
        pending.append(lambda: normalize(state["csbB"], hp, qc, slice(HD, P)))

    def outproj(st):
        ps = psum.tile([P, D], F32, tag="sc", bufs=2)
        for half in range(2):
            sl = slice(half * 512, (half + 1) * 512)
            for c in range(4):
                nc.tensor.matmul(
                    ps[:, sl],
                    lhsT=ctxt[c][:, st * P:(st + 1) * P],
                    rhs=wout_bf[c][:, sl],
                    start=(c == 0),
                    stop=(c == 3),
                )
        osb = outsb_pool.tile([P, D], F32, tag="osb", bufs=3)
        nc.vector.tensor_copy(osb[:], ps[:])
        eng = (nc.gpsimd, nc.sync, nc.scalar)[st % 3]
        eng.dma_start(out_ap[st * P:(st + 1) * P, :], osb[:])

    for qc in range(2):
        for hp in range(4):
            attend(hp, qc)
    drain(len(pending))
    for st in range(NST):
        outproj(st)


_CACHED = None


def _get_nc():
    global _CACHED
    if _CACHED is None:
        nc = bacc.Bacc(
            "TRN2", target_bir_lowering=False, debug=False, num_devices=8
        )
        hs = nc.dram_tensor("hs", [S, D], F32, kind="ExternalInput").ap()
        wqkv = nc.dram_tensor("wqkv", [D, 3 * DQ], F32, kind="ExternalInput").ap()
        wout = nc.dram_tensor("wout", [DQ, D], F32, kind="ExternalInput").ap()
        out = nc.dram_tensor("out", [S, D], F32, kind="ExternalOutput").ap()
        build_kernel(nc, out, hs, wqkv, wout)
        nc.compile()
        _CACHED = nc
    return _CACHED


def make_in_maps(hidden_states, w_qkv, w_out):
    in_maps = []
    for c in range(8):
        b, g = divmod(c, 2)
        cols = slice(g * DQ, (g + 1) * DQ)
        wq = w_qkv[:, 0 * D:1 * D][:, cols]
        wk = w_qkv[:, 1 * D:2 * D][:, cols]
        wv = w_qkv[:, 2 * D:3 * D][:, cols]
        in_maps.append({
            "hs": np.ascontiguousarray(hidden_states[b], dtype=np.float32),
            "wqkv": np.ascontiguousarray(
                np.concatenate([wq, wk, wv], axis=1), dtype=np.float32
            ),
            "wout": np.ascontiguousarray(
                w_out[g * DQ:(g + 1) * DQ, :], dtype=np.float32
            ),
        })
    return in_maps


def run(hidden_states, w_qkv, w_out, trace=False):
    nc = _get_nc()
    in_maps = make_in_maps(hidden_states, w_qkv, w_out)
    res = run_bass_kernel_spmd(nc, in_maps, core_ids=list(range(8)), trace=trace)
    out = np.empty((4, S, D), np.float32)
    for b in range(4):
        out[b] = res.results[2 * b]["out"] + res.results[2 * b + 1]["out"]
    return out, res


def kernel(hidden_states, w_qkv, w_out):
    out, _ = run(
        np.asarray(hidden_states), np.asarray(w_qkv), np.asarray(w_out)
    )
    return out


# revision 19
# speedup vs baseline: 1.2074x; 1.2074x over previous
"""AIMv2 attention (B=4, S=2048, D=1024, H=16, d=64) on 8 TRN2 NeuronCores.

Sharding: core c = (batch b = c//2, head-group g = c%2 of 8 heads).
Each core computes its batch's attention for its 8 heads plus the
out-projection partial sum over its heads' rows of w_out; the host adds
the two partials per batch (no on-device collectives needed).

Per-core kernel (all matmuls in bf16, fp32 accumulation):
  X^T via DVE cast + batched XBAR DMA transposes; Q^T,K^T = Wq/k^T @ X^T
  so the score matmuls produce s_T[k, q] directly with head pairs in
  row-groups (even head partitions 0-63, odd 64-127); softmax without
  max-subtraction (scores ~ N(0,1), exp never overflows); V carries a
  ones column so ctx' = [V|1]^T @ P^T yields both ctx^T and the softmax
  denominators in one accumulation.

  The attention inner loop is a lag-3 software pipeline: ctx matmuls for
  k-tile kt are emitted alongside scores for kt+3, so TensorE never
  waits on the ScalarE exp stream (the kernel is exp-throughput bound).
  Cross-pair cleanup (last ctx tiles, PSUM evacuation, normalization)
  drains during the first k-tiles of the next pair.
"""

import numpy as np

import concourse.bass as bass
import concourse.tile as tile
from concourse import bacc, mybir
from concourse.bass_utils import run_bass_kernel_spmd

P = 128
S = 2048          # sequence length
D = 1024          # model dim
DQ = 512          # per-core qkv width (8 heads x 64)
HD = 64           # head dim
NH = 8            # heads per core
NKT = D // P      # 8 contraction tiles over D
NST = S // P      # 16 tiles over S
QC = 1024         # q chunk for attention inner loop
LAG = 4           # ctx matmul lag behind scores/exp in the pipeline
SCALE = 1.0 / 8.0  # 1/sqrt(64)

F32 = mybir.dt.float32
BF16 = mybir.dt.bfloat16


def build_kernel(nc, out_ap, hs_ap, wqkv_ap, wout_ap):
    import contextlib

    ctx = contextlib.ExitStack()
    with tile.TileContext(nc) as tc:
        with ctx:
            _body(ctx, tc, nc, out_ap, hs_ap, wqkv_ap, wout_ap)


def _body(ctx, tc, nc, out_ap, hs_ap, wqkv_ap, wout_ap):
    Exp = mybir.ActivationFunctionType.Exp

    persist = ctx.enter_context(tc.tile_pool(name="persist", bufs=1))
    psum = ctx.enter_context(tc.tile_pool(name="psum", bufs=1, space="PSUM"))

    # all-ones [128, 64] so a ones-row lhsT can be sliced at any base
    # partition (matmul requires lhsT/rhs base partitions to match)
    ones_rows = persist.tile([P, HD], BF16, name="ones_rows")
    nc.vector.memset(ones_rows[:], 1.0)

    wout_bf = []
    vc = [persist.tile([P, NH, HD + 1], BF16, name=f"vc{st}") for st in range(NST)]
    qt = [persist.tile([P, S], BF16, name=f"qt{m}") for m in range(4)]
    kt_sb = [persist.tile([P, S], BF16, name=f"kt{m}") for m in range(4)]
    ctxt = [persist.tile([P, S], BF16, name=f"ctxt{m}") for m in range(4)]

    # ================= head: loads + all projections =================
    # X^T and the qkv weights are only needed for the projections; scope
    # them so their SBUF goes back before the attention pools open.
    with tc.tile_pool(name="proj_scope", bufs=1) as pscope:
        xt3 = pscope.tile([P, NKT, S], BF16, name="xt3")
        wqkv_bf = []
        with tc.tile_pool(name="stage", bufs=3) as stage:
            for kt in range(NKT):
                wf = stage.tile([P, 3 * DQ], F32, tag="wstage", bufs=2)
                nc.gpsimd.dma_start(wf[:], wqkv_ap[kt * P:(kt + 1) * P, :])
                wb = pscope.tile([P, 3 * DQ], BF16, name=f"wqkv_bf{kt}")
                nc.vector.tensor_copy(wb[:], wf[:])
                wqkv_bf.append(wb)

            for i in range(DQ // P):
                wf = stage.tile([P, D], F32, tag="wostage", bufs=2)
                nc.gpsimd.dma_start(wf[:], wout_ap[i * P:(i + 1) * P, :])
                wb = persist.tile([P, D], BF16, name=f"wout_bf{i}")
                nc.vector.tensor_copy(wb[:], wf[:])
                wout_bf.append(wb)

            warm_ps = psum.tile([HD, HD], F32, tag="ctxA", bufs=1, name="warm_ps")
            n_warm = 2 * NST
            wi = 0
            def warm(src_tile):
                nonlocal wi
                nc.tensor.matmul(
                    warm_ps[:], lhsT=src_tile[0:HD, 0:HD],
                    rhs=src_tile[0:HD, 0:HD],
                    start=(wi == 0), stop=(wi == n_warm - 1),
                )
                wi += 1

            # X: load, cast bf16, batched XBAR transpose into [D, S] layout
            for st in range(NST):
                xf = stage.tile([P, D], F32, tag="xstage", bufs=5)
                nc.scalar.dma_start(xf[:], hs_ap[st * P:(st + 1) * P, :])
                xb = stage.tile([P, D], BF16, tag="xbf", bufs=4)
                nc.vector.tensor_copy(xb[:], xf[:])
                nc.sync.dma_start_transpose(
                    xt3[:, :, st * P:(st + 1) * P], xb[:]
                )
                warm(xf)
                warm(xb)

        warmsb = pscope.tile([HD, HD], F32, name="warmsb")
        nc.vector.tensor_scalar_mul(warmsb[:], warm_ps[:], 0.0)
        nc.vector.tensor_add(ones_rows[0:HD, :], ones_rows[0:HD, :], warmsb[:])

        def xt(kt):
            return xt3[:, kt, :]

        # V projection with ones column: vc[st][:, h, 0:64]=V_h, [...,64]=1
        for st in range(NST):
            nc.vector.memset(vc[st][:, :, HD:HD + 1], 1.0)
        for stq in range(NST // 2):
            ps = psum.tile([P, 2 * DQ], F32, tag="sc", bufs=2)
            for half in range(2):
                st = 2 * stq + half
                sl = slice(half * DQ, (half + 1) * DQ)
                for kt in range(NKT):
                    nc.tensor.matmul(
                        ps[:, sl],
                        lhsT=xt(kt)[:, st * P:(st + 1) * P],
                        rhs=wqkv_bf[kt][:, 2 * DQ:3 * DQ],
                        start=(kt == 0),
                        stop=(kt == NKT - 1),
                    )
            for half in range(2):
                st = 2 * stq + half
                src = ps[:, half * DQ:(half + 1) * DQ].rearrange(
                    "p (h e) -> p h e", h=NH
                )
                nc.vector.tensor_copy(vc[st][:, :, 0:HD], src)

        # Q^T / K^T projections for all head pairs
        for m in range(4):
            for which, dst in ((0, qt[m]), (DQ, kt_sb[m])):
                for nqq in range(2):
                    ps = psum.tile([P, 2 * 512], F32, tag="sc", bufs=2)
                    for half in range(2):
                        nq = 2 * nqq + half
                        sl = slice(half * 512, (half + 1) * 512)
                        for kt in range(NKT):
                            nc.tensor.matmul(
                                ps[:, sl],
                                lhsT=wqkv_bf[kt][
                                    :, which + m * P: which + (m + 1) * P
                                ],
                                rhs=xt(kt)[:, nq * 512:(nq + 1) * 512],
                                start=(kt == 0),
                                stop=(kt == NKT - 1),
                            )
                    nc.vector.tensor_copy(
                        dst[:, nqq * 1024:(nqq + 1) * 1024], ps[:]
                    )

    # ================= attention =================
    pt_pool = ctx.enter_context(tc.tile_pool(name="pt", bufs=18))
    small = ctx.enter_context(tc.tile_pool(name="small", bufs=4))
    outsb_pool = ctx.enter_context(tc.tile_pool(name="outsb", bufs=3))

    # deferred cross-pair work: closures drained 2-per-k-tile during the
    # first LAG k-tiles of the following pair (while it has no ctx work)
    pending = []

    def drain(n):
        for _ in range(min(n, len(pending))):
            pending.pop(0)()

    def normalize(csb, hp, qc, rows):
        """ctx^T[d,q] /= sum[q] (sums in row 64 of csb)."""
        q0 = qc * QC
        bc = psum.tile([HD, QC], F32, tag="sc", bufs=2)
        for half in range(2):
            sl = slice(half * 512, (half + 1) * 512)
            nc.tensor.matmul(
                bc[:, sl], lhsT=ones_rows[HD:HD + 1, :],
                rhs=csb[HD:HD + 1, sl],
                start=True, stop=True,
            )
        rec = small.tile([HD, QC], F32, tag="rec", bufs=2)
        nc.vector.reciprocal_approx_fast(rec[:], bc[:])
        nc.vector.tensor_mul(
            ctxt[hp][rows, q0:q0 + QC], csb[0:HD, :], rec[:]
        )

    def attend(hp, qc):
        """Heads (2hp, 2hp+1): even head on partitions 0-63, odd on 64-127."""
        q0 = qc * QC
        hA, hB = 2 * hp, 2 * hp + 1
        state = {}

        def emit_scores(kti):
            psA = psum.tile([P, QC], F32, tag="sc", bufs=2)
            psB = psum.tile([P, QC], F32, tag="sc", bufs=2)
            for half in range(2):
                sl = slice(half * 512, (half + 1) * 512)
                qsl = slice(q0 + half * 512, q0 + (half + 1) * 512)
                nc.tensor.matmul(
                    psA[:, sl],
                    lhsT=kt_sb[hp][0:HD, kti * P:(kti + 1) * P],
                    rhs=qt[hp][0:HD, qsl],
                    start=True, stop=True,
                )
                nc.tensor.matmul(
                    psB[:, sl],
                    lhsT=kt_sb[hp][HD:P, kti * P:(kti + 1) * P],
                    rhs=qt[hp][HD:P, qsl],
                    start=True, stop=True,
                )
            return psA, psB

        def emit_exp(psA, psB):
            ptA = pt_pool.tile([P, QC], BF16, tag="pt", bufs=18)
            ptB = pt_pool.tile([P, QC], BF16, tag="pt", bufs=18)
            nc.scalar.activation(ptA[:], psA[:], Exp, scale=SCALE)
            nc.scalar.activation(ptB[:], psB[:], Exp, scale=SCALE)
            return ptA, ptB

        def emit_ctx(kti, ptA, ptB):
            if kti == 0:
                state["ctxA"] = psum.tile([HD + 1, QC], F32, tag="ctxA", bufs=1, name="ctxA")
                state["ctxB"] = psum.tile([HD + 1, QC], F32, tag="ctxB", bufs=1, name="ctxB")
            first = kti == 0
            last = kti == NST - 1
            for half in range(2):
                sl = slice(half * 512, (half + 1) * 512)
                nc.tensor.matmul(
                    state["ctxA"][:, sl], lhsT=vc[kti][:, hA, :],
                    rhs=ptA[:, sl], start=first, stop=last,
                )
                nc.tensor.matmul(
                    state["ctxB"][:, sl], lhsT=vc[kti][:, hB, :],
                    rhs=ptB[:, sl], start=first, stop=last,
                )

        pts = {}
        for kti in range(NST):
            ps = emit_scores(kti)
            if kti < LAG:
                drain(2)           # previous pair's tail work
            else:
                emit_ctx(kti - LAG, *pts.pop(kti - LAG))
            pts[kti] = emit_exp(*ps)

        # tail: last LAG ctx tiles + PSUM evacuation + normalization are
        # deferred into the next pair's first k-tiles
        def tail_ctx(kti):
            def f():
                emit_ctx(kti, *pts.pop(kti))
            return f

        for kti in range(NST - LAG, NST):
            pending.append(tail_ctx(kti))

        def evac():
            csbA = small.tile([HD + 1, QC], BF16, tag="csb", bufs=4)
            nc.vector.tensor_copy(csbA[:], state["ctxA"][:])
            csbB = small.tile([HD + 1, QC], BF16, tag="csb", bufs=4)
            nc.vector.tensor_copy(csbB[:], state["ctxB"][:])
            state["csbA"], state["csbB"] = csbA, csbB

        pending.append(evac)
        pending.append(lambda: normalize(state["csbA"], hp, qc, slice(0, HD)))
        pending.append(lambda: normalize(state["csbB"], hp, qc, slice(HD, P)))

    def outproj(st):
        ps = psum.tile([P, D], F32, tag="sc", bufs=2)
        for half in range(2):
            sl = slice(half * 512, (half + 1) * 512)
            for c in range(4):
                nc.tensor.matmul(
                    ps[:, sl],
                    lhsT=ctxt[c][:, st * P:(st + 1) * P],
                    rhs=wout_bf[c][:, sl],
                    start=(c == 0),
                    stop=(c == 3),
                )
        osb = outsb_pool.tile([P, D], F32, tag="osb", bufs=3)
        if st % 2 == 0:
            nc.vector.tensor_copy(osb[:], ps[:])
        else:
            nc.scalar.copy(osb[:], ps[:])
        eng = (nc.gpsimd, nc.sync)[st % 2]
        eng.dma_start(out_ap[st * P:(st + 1) * P, :], osb[:])

    for qc in range(2):
        for hp in range(4):
            attend(hp, qc)
    drain(len(pending))
    for st in range(NST):
        outproj(st)


_CACHED = None


def _get_nc():
    global _CACHED
    if _CACHED is None:
        nc = bacc.Bacc(
            "TRN2", target_bir_lowering=False, debug=False, num_devices=8
        )
        hs = nc.dram_tensor("hs", [S, D], F32, kind="ExternalInput").ap()
        wqkv = nc.dram_tensor("wqkv", [D, 3 * DQ], F32, kind="ExternalInput").ap()
        wout = nc.dram_tensor("wout", [DQ, D], F32, kind="ExternalInput").ap()
        out = nc.dram_tensor("out", [S, D], F32, kind="ExternalOutput").ap()
        build_kernel(nc, out, hs, wqkv, wout)
        nc.compile()
        _CACHED = nc
    return _CACHED


def make_in_maps(hidden_states, w_qkv, w_out):
    in_maps = []
    for c in range(8):
        b, g = divmod(c, 2)
        cols = slice(g * DQ, (g + 1) * DQ)
        wq = w_qkv[:, 0 * D:1 * D][:, cols]
        wk = w_qkv[:, 1 * D:2 * D][:, cols]
        wv = w_qkv[:, 2 * D:3 * D][:, cols]
        in_maps.append({
            "hs": np.ascontiguousarray(hidden_states[b], dtype=np.float32),
            "wqkv": np.ascontiguousarray(
                np.concatenate([wq, wk, wv], axis=1), dtype=np.float32
            ),
            "wout": np.ascontiguousarray(
                w_out[g * DQ:(g + 1) * DQ, :], dtype=np.float32
            ),
        })
    return in_maps


def run(hidden_states, w_qkv, w_out, trace=False):
    nc = _get_nc()
    in_maps = make_in_maps(hidden_states, w_qkv, w_out)
    res = run_bass_kernel_spmd(nc, in_maps, core_ids=list(range(8)), trace=trace)
    out = np.empty((4, S, D), np.float32)
    for b in range(4):
        out[b] = res.results[2 * b]["out"] + res.results[2 * b + 1]["out"]
    return out, res


def kernel(hidden_states, w_qkv, w_out):
    out, _ = run(
        np.asarray(hidden_states), np.asarray(w_qkv), np.asarray(w_out)
    )
    return out
